# revision 16
# baseline (speedup 1.0000x reference)
"""Multi-head causal self-attention (B=4, T=2048, D=1024, H=16) on 8 TRN2
NeuronCores.

Sharding: core c handles batch b = c//2 and half the heads (8 heads = 512
local dims).  Each core runs an identical Bass/Tile NEFF (SPMD, no
collectives) computing:

    Q^T = (s*Wq_slice) @ x_q^T          (512, 2048)  [spilled to DRAM]
    K^T = Wk_slice @ x_k^T              (512, 2048)  [SBUF resident]
    V   = x_v @ Wv_slice^T              (2048, 512)  [SBUF, +ones col/head]
    per (q-block, head):  S^T = K^T_chunk.T-matmuls, exp, P^T V via PE,
                          softmax denominator from an appended ones column
    out_partial = ctx @ Wo[:, slice].T  (2048, 1024)

The host sums the two partial outputs per batch (row-parallel output
projection) and adds the output bias.

Score scale 1/sqrt(64) is folded into Wq on the host.  bq/bk/bv are zero
for this problem's deterministic inputs; a numpy fallback covers the
general case.
"""

from contextlib import ExitStack

import numpy as np

import concourse.bass as bass
import concourse.tile as tile
from concourse import bass_utils, mybir
from concourse.tile_sem_assignment import N_PROCS
from concourse.vector_clock import ScopedClock, VectorClock

F32 = mybir.dt.float32
F32R = mybir.dt.float32r

P = 128          # partition dim
T = 2048         # sequence length
DIN = 1024       # model dim
DLOC = 512       # local head dims per core (8 heads x 64)
NHL = 8          # local heads per core
DK = 64          # head dim
VSLOT = DK + 1   # V columns per head incl. the denominator ones column
NQ = 512         # q-block width (one fp32 PSUM bank)
KC = DIN // P    # 8  contraction chunks for projections
NT = T // NQ     # 4  t-blocks of 512
NTC = T // P     # 16 t-chunks of 128
NEG = -1.0e30
N_CORES = 8
EXP = mybir.ActivationFunctionType.Exp


class _SplitDrainTileContext(tile.TileContext):
    """Workaround: the walrus build in this container rejects a Drain
    instruction carrying more than a couple of sync waits ("Too many sync
    wait commands").  Emit one Drain per logical proc instead of the stock
    single Drain with one wait per proc."""

    def _drain_and_barrier(self, tick_clock, wait_clock):
        gc = tick_clock.global_clock
        for p in range(N_PROCS):
            if gc[p] > 0:
                sub = VectorClock([gc[q] if q == p else 0 for q in range(N_PROCS)])
                drain_inst = self.nc.sync.drain()
                wait_clock.add_sem_waits(drain_inst.ins, ScopedClock({None: sub}))
        self.nc.all_engine_barrier()
        assert self.sems is not None
        popped = self.nc._tile_sem_poison_stack.pop()
        assert popped is self._sem_poison
        self.nc.clear_and_free_semaphores(list(self.sems.allocated().values()))
        self.nc.all_engine_barrier()


_MAX_WAITS = 1  # this walrus build rejects instructions with more sync waits


def _split_excess_waits(nc: bass.Bass, max_waits: int = _MAX_WAITS) -> None:
    """Move sync waits beyond `max_waits` per instruction onto preceding
    single-wait EventSemaphore instructions on the same engine (same engine
    queue => executes first, so semantics are preserved)."""
    n = 0
    for f in nc.m.functions:
        for b in f.blocks:
            out = []
            changed = False
            for inst in b.instructions:
                si = inst.sync_info
                waits = list(si.on_wait) if si is not None and si.on_wait else []
                if len(waits) > max_waits:
                    for w in waits[:-max_waits]:
                        n += 1
                        out.append(
                            mybir.InstEventSemaphore(
                                name=f"xsplitw_{n}",
                                engine=inst.engine,
                                ins=[],
                                outs=[],
                                sync_info=mybir.SyncInfo(on_wait=[w], on_update=[]),
                            )
                        )
                    inst.sync_info = mybir.SyncInfo(
                        on_wait=waits[-max_waits:], on_update=list(si.on_update)
                    )
                    changed = True
                out.append(inst)
            if changed:
                b.instructions = out


def _build_program() -> bass.Bass:
    nc = bass.Bass(trn_type="TRN2", debug=False, num_devices=N_CORES)

    xq_d = nc.dram_tensor("xq", [DIN, T], F32R, kind="ExternalInput").ap()
    xk_d = nc.dram_tensor("xk", [DIN, T], F32R, kind="ExternalInput").ap()
    xv_d = nc.dram_tensor("xv", [DIN, T], F32R, kind="ExternalInput").ap()
    wq_d = nc.dram_tensor("wq", [DIN, DLOC], F32R, kind="ExternalInput").ap()
    wk_d = nc.dram_tensor("wk", [DIN, DLOC], F32R, kind="ExternalInput").ap()
    wv_d = nc.dram_tensor("wv", [DIN, DLOC], F32R, kind="ExternalInput").ap()
    wo_d = nc.dram_tensor("wo", [DLOC, DIN], F32R, kind="ExternalInput").ap()
    mask_d = nc.dram_tensor("mask", [P, P], F32, kind="ExternalInput").ap()
    out_d = nc.dram_tensor("out", [T, DIN], F32, kind="ExternalOutput").ap()
    rt_d = nc.dram_tensor("rt_spill", [NT * NHL, NQ], F32).ap()

    with nc.allow_low_precision(
        reason="fp32r matmuls: 4x PE throughput, ~2e-4 rel err"
    ), _SplitDrainTileContext(nc) as tc, ExitStack() as ctx:
        persist = ctx.enter_context(tc.tile_pool(name="persist", bufs=1))
        wpool = ctx.enter_context(tc.tile_pool(name="w", bufs=10))
        wopool = ctx.enter_context(tc.tile_pool(name="wo", bufs=8))
        xpool = ctx.enter_context(tc.tile_pool(name="x", bufs=8))
        stage = ctx.enter_context(tc.tile_pool(name="stage", bufs=4))
        qrpool = ctx.enter_context(tc.tile_pool(name="qr", bufs=9))
        epool = ctx.enter_context(tc.tile_pool(name="e", bufs=4))
        rpool = ctx.enter_context(tc.tile_pool(name="r", bufs=2))
        rbpool = ctx.enter_context(tc.tile_pool(name="rb", bufs=2))
        cxpool = ctx.enter_context(tc.tile_pool(name="cx", bufs=8))
        ps_pp = ctx.enter_context(tc.tile_pool(name="ps_pp", bufs=2, space="PSUM"))
        ps_s = ctx.enter_context(tc.tile_pool(name="ps_s", bufs=2, space="PSUM"))
        ps_ctx = ctx.enter_context(tc.tile_pool(name="ps_ctx", bufs=2, space="PSUM"))

        # ---- persistent SBUF buffers ----
        kt = [persist.tile([P, T], F32R, name=f"kt{i}", tag=f"kt{i}") for i in range(4)]
        va = persist.tile([P, NTC * NHL * VSLOT], F32R, name="va", tag="va")
        mask_sb = persist.tile([P, P], F32, name="mask_sb", tag="mask")

        nc.sync.dma_start(out=mask_sb, in_=mask_d)
        # memset through an f32 bitcast view: f32r memset fails an ISA check
        # in this walrus build, and 1.0 has identical bits in both formats
        va_view = va.rearrange("p (t h e) -> p t h e", h=NHL, e=VSLOT)
        va_view_f32 = va.bitcast(F32).rearrange(
            "p (t h e) -> p t h e", h=NHL, e=VSLOT
        )
        nc.vector.memset(va_view_f32[:, :, :, DK : DK + 1], 1.0)

        # ================= projection chunk builders =================
        # Each block b = {V(tg=b), Q(nt=b), K(nt=b)} is emitted as small
        # chunks interleaved into the attention instruction stream so the PE
        # always has filler work while ACT chews through exp tiles.

        def v_chunks(tg):
            st = {}

            def c_dma(part):
                if part == 0:
                    st["w"] = []
                    st["x"] = []
                for kc in range(part * 2, part * 2 + 2):
                    wt = wpool.tile([P, DLOC], F32R, name=f"wv{tg}_{kc}", tag="w")
                    nc.sync.dma_start(out=wt, in_=wv_d[kc * P : (kc + 1) * P, :])
                    st["w"].append(wt)
                    xc = xpool.tile([P, NQ], F32R, name=f"xv{tg}_{kc}", tag="x")
                    nc.sync.dma_start(
                        out=xc,
                        in_=xv_d[kc * P : (kc + 1) * P, tg * NQ : (tg + 1) * NQ],
                    )
                    st["x"].append(xc)

            def c_half(half):
                psums = [
                    ps_pp.tile([P, DLOC], F32, name=f"vps{tg}_{half}_{i}", tag="pp")
                    for i in range(2)
                ]
                for kc in range(KC):
                    for i in range(2):
                        tsub = half * 2 + i
                        nc.tensor.matmul(
                            psums[i],
                            lhsT=st["x"][kc][:, tsub * P : (tsub + 1) * P],
                            rhs=st["w"][kc],
                            start=(kc == 0),
                            stop=(kc == KC - 1),
                        )
                for i in range(2):
                    tci = tg * 4 + half * 2 + i
                    nc.vector.tensor_copy(
                        out=va_view[:, tci, :, 0:DK],
                        in_=psums[i].rearrange("p (h e) -> p h e", e=DK),
                    )

            return [lambda p=p: c_dma(p) for p in range(4)] + [
                lambda: c_half(0),
                lambda: c_half(1),
            ]

        def qk_chunks(nt, w_dram, x_dram, sink, label):
            st = {}

            def c_dma(part):
                if part == 0:
                    st["w"] = []
                    st["x"] = []
                for kc in range(part * 2, part * 2 + 2):
                    wt = wpool.tile([P, DLOC], F32R, name=f"w{label}{nt}_{kc}", tag="w")
                    nc.sync.dma_start(out=wt, in_=w_dram[kc * P : (kc + 1) * P, :])
                    st["w"].append(wt)
                    xc = xpool.tile([P, NQ], F32R, name=f"x{label}{nt}_{kc}", tag="x")
                    nc.sync.dma_start(
                        out=xc,
                        in_=x_dram[kc * P : (kc + 1) * P, nt * NQ : (nt + 1) * NQ],
                    )
                    st["x"].append(xc)

            def c_half(mh):
                psums = [
                    ps_pp.tile([P, NQ], F32, name=f"{label}ps{nt}_{mh}_{i}", tag="pp")
                    for i in range(2)
                ]
                for kc in range(KC):
                    for i in range(2):
                        mq = mh * 2 + i
                        nc.tensor.matmul(
                            psums[i],
                            lhsT=st["w"][kc][:, mq * P : (mq + 1) * P],
                            rhs=st["x"][kc],
                            start=(kc == 0),
                            stop=(kc == KC - 1),
                        )
                for i in range(2):
                    sink(mh * 2 + i, nt, psums[i])

            return [lambda p=p: c_dma(p) for p in range(4)] + [
                lambda: c_half(0),
                lambda: c_half(1),
            ]

        qt_sb = {}

        def q_sink(mq, nt, psum):
            qt = qrpool.tile([P, NQ], F32R, name=f"qt{nt}_{mq}", tag="qr")
            nc.vector.tensor_copy(out=qt, in_=psum)
            qt_sb[(nt, mq)] = qt

        def k_sink(mq, nt, psum):
            nc.vector.tensor_copy(out=kt[mq][:, nt * NQ : (nt + 1) * NQ], in_=psum)

        def block_chunks(b):
            return (
                v_chunks(b)
                + qk_chunks(b, wq_d, xq_d, q_sink, "q")
                + qk_chunks(b, wk_d, xk_d, k_sink, "k")
            )

        # ctxn[(qi, hp)]: normalized ctx^T rows [hp*128,+128) x cols qi-block
        ctxn = {}
        wo_sb = {}

        def load_wo():
            for kc4 in range(4):
                for n in range(2):
                    wt = wopool.tile([P, NQ], F32R, name=f"wo{kc4}_{n}", tag="wo")
                    nc.sync.dma_start(
                        out=wt,
                        in_=wo_d[kc4 * P : (kc4 + 1) * P, n * NQ : (n + 1) * NQ],
                    )
                    wo_sb[(kc4, n)] = wt

        def op_chunk(qi, tsub, n):
            tci = qi * 4 + tsub

            def c():
                ops = ps_pp.tile([P, NQ], F32, name=f"ops{tci}_{n}", tag="pp")
                for kc4 in range(4):
                    nc.tensor.matmul(
                        ops,
                        lhsT=ctxn[(qi, kc4)][:, tsub * P : (tsub + 1) * P],
                        rhs=wo_sb[(kc4, n)],
                        start=(kc4 == 0),
                        stop=(kc4 == 3),
                    )
                st = stage.tile([P, NQ], F32, name=f"ost{tci}_{n}", tag="stage")
                nc.vector.tensor_copy(out=st, in_=ops)
                nc.sync.dma_start(
                    out=out_d[tci * P : (tci + 1) * P, n * NQ : (n + 1) * NQ],
                    in_=st,
                )

            return c

        # ================= filler scheduler =================
        fill = []  # list of (block_id_or_None, closure)
        for b in (1, 2, 3):
            fill.extend((b, c) for c in block_chunks(b))
        blocks_left = {1: 18, 2: 18, 3: 18}

        def pump(n=1):
            for _ in range(n):
                if not fill:
                    return
                b, c = fill.pop(0)
                c()
                if b is not None:
                    blocks_left[b] -= 1

        def ensure_blocks(qi):
            while any(blocks_left.get(b, 0) > 0 for b in range(1, qi + 1)):
                pump(1)

        # ================= prologue: block 0 =================
        for c in block_chunks(0):
            c()
        load_wo()

        # ================= attention + interleaved filler =================
        def ctx_mm2(hp, sub, et, jp, jmax, cps, qi):
            h = 2 * hp + sub
            for jj in range(2):
                j = 2 * jp + jj
                off = max(0, j * P - qi * NQ)
                base = jj * NQ
                nc.tensor.matmul(
                    cps[sub] if j == 0 else cps[sub][:, off:NQ],
                    lhsT=va_view[:, j, h, :],
                    rhs=et[:, base : base + NQ] if j == 0 else et[:, base + off : base + NQ],
                    start=(j == 0),
                    stop=(j == jmax - 1),
                    skip_group_check=True,
                )

        step = 0
        credit = 0.0
        for qi in range(NT):
            ensure_blocks(qi)
            # pump cadence: finish all projection blocks well before the
            # ACT-bound final q-block; only out-proj chunks remain for qi=3
            pump_plan = {0: 1.0, 1: 1.5, 2: 1.5, 3: 0.25}[qi]
            jmax = 4 * (qi + 1)
            for hp in range(NHL // 2):
                ctxn[(qi, hp)] = cxpool.tile(
                    [P, NQ], F32R, name=f"ctxn{qi}_{hp}", tag="cx"
                )
                qt_t = qt_sb[(qi, hp)]
                cps = [
                    ps_ctx.tile([VSLOT, NQ], F32, name=f"cps{qi}_{hp}_{s}", tag="ctx")
                    for s in range(2)
                ]
                pend = []  # [(sub, et, jp)]
                for jp in range(jmax // 2):
                    j0, j1 = 2 * jp, 2 * jp + 1
                    d0 = j0 * P - qi * NQ
                    d1 = j1 * P - qi * NQ
                    off0, off1 = max(0, d0), max(0, d1)
                    cur = []
                    for sub in range(2):
                        h = 2 * hp + sub
                        krow = sub * DK
                        # two j-chunks share one 2-bank PSUM tile so one ACT
                        # instruction exponentiates both (halves ACT overhead)
                        sps = ps_s.tile(
                            [P, 2 * NQ], F32, name=f"sps{qi}_{h}_{jp}", tag="s"
                        )
                        nc.tensor.matmul(
                            sps[:, off0:NQ],
                            lhsT=kt[hp][krow : krow + DK, j0 * P : (j0 + 1) * P],
                            rhs=qt_t[krow : krow + DK, off0:NQ],
                            start=True,
                            stop=True,
                        )
                        nc.tensor.matmul(
                            sps[:, NQ + off1 : 2 * NQ],
                            lhsT=kt[hp][krow : krow + DK, j1 * P : (j1 + 1) * P],
                            rhs=qt_t[krow : krow + DK, off1:NQ],
                            start=True,
                            stop=True,
                        )
                        cur.append((sub, sps))
                    for (sub, et, pjp) in pend:
                        ctx_mm2(hp, sub, et, pjp, jmax, cps, qi)
                    pend = []
                    for (sub, sps) in cur:
                        h = 2 * hp + sub
                        if d0 >= 0:
                            nc.vector.tensor_add(
                                sps[:, off0 : off0 + P], sps[:, off0 : off0 + P], mask_sb
                            )
                        if d1 >= 0:
                            nc.vector.tensor_add(
                                sps[:, NQ + off1 : NQ + off1 + P],
                                sps[:, NQ + off1 : NQ + off1 + P],
                                mask_sb,
                            )
                        et = epool.tile(
                            [P, 2 * NQ], F32R, name=f"et{qi}_{h}_{jp}", tag="e"
                        )
                        nc.scalar.activation(
                            out=et[:, off0 : 2 * NQ], in_=sps[:, off0 : 2 * NQ], func=EXP
                        )
                        pend.append((sub, et, jp))
                    step += 1
                    credit = credit + pump_plan
                    while credit >= 1.0:
                        pump(1)
                        credit -= 1.0
                for (sub, et, pjp) in pend:
                    ctx_mm2(hp, sub, et, pjp, jmax, cps, qi)

                # normalize ctx[dv, q] by 1/denom[q]; the (1,q) reciprocal row
                # is broadcast across 64 partitions via a DRAM round-trip
                for sub in range(2):
                    h = 2 * hp + sub
                    krow = sub * DK
                    idx = qi * NHL + h
                    rt = rpool.tile([1, NQ], F32, name=f"rt{qi}_{h}", tag="recip")
                    nc.vector.reciprocal(rt, cps[sub][DK : DK + 1, :])
                    nc.sync.dma_start(out=rt_d[idx : idx + 1, :], in_=rt)
                    rb = rbpool.tile([DK, NQ], F32, name=f"rb{qi}_{h}", tag="rb")
                    nc.gpsimd.dma_start(
                        out=rb,
                        in_=bass.AP(
                            tensor=rt_d.tensor,
                            offset=idx * NQ,
                            ap=[[0, DK], [1, NQ]],
                        ),
                    )
                    nc.vector.tensor_mul(
                        ctxn[(qi, hp)][krow : krow + DK, :], cps[sub][0:DK, :], rb
                    )
            # this q-block's output projection becomes late filler
            for tsub in range(4):
                for n in range(2):
                    fill.append((None, op_chunk(qi, tsub, n)))

        # drain remaining filler (late out-projection chunks)
        while fill:
            pump(1)

    _split_excess_waits(nc)
    return nc


_NC_CACHE: bass.Bass | None = None


def _get_program() -> bass.Bass:
    global _NC_CACHE
    if _NC_CACHE is None:
        _NC_CACHE = _build_program()
    return _NC_CACHE


def _numpy_reference(q, k, v, Wq, Wk, Wv, Wo, bq, bk, bv, bo):
    """Exact fallback, used only if bq/bk/bv are nonzero (never the case for
    this problem's deterministic inputs)."""
    B, T_, D = q.shape
    H = 16
    dk = D // H

    def split(x):
        return x.reshape(B, T_, H, dk).transpose(0, 2, 1, 3)

    qh = split(q @ Wq.T + bq)
    kh = split(k @ Wk.T + bk)
    vh = split(v @ Wv.T + bv)
    scores = np.einsum("bhqd,bhkd->bhqk", qh, kh) / np.sqrt(np.float32(dk))
    causal = np.tril(np.ones((T_, T_), dtype=bool))
    scores = np.where(causal, scores, -np.inf).astype(np.float32)
    scores -= scores.max(axis=-1, keepdims=True)
    e = np.exp(scores)
    attn = e / e.sum(axis=-1, keepdims=True)
    ctx = np.einsum("bhqk,bhkd->bhqd", attn, vh)
    merged = ctx.transpose(0, 2, 1, 3).reshape(B, T_, D)
    return (merged @ Wo.T + bo).astype(np.float32)


def kernel(q, k, v, Wq, Wk, Wv, Wo, bq, bk, bv, bo):
    q, k, v = (np.asarray(a, np.float32) for a in (q, k, v))
    Wq, Wk, Wv, Wo = (np.asarray(a, np.float32) for a in (Wq, Wk, Wv, Wo))
    bq, bk, bv, bo = (np.asarray(a, np.float32) for a in (bq, bk, bv, bo))

    if np.any(bq) or np.any(bk) or np.any(bv):
        return _numpy_reference(q, k, v, Wq, Wk, Wv, Wo, bq, bk, bv, bo)

    B = q.shape[0]
    scale = np.float32(1.0 / np.sqrt(DK))
    wq_s = (Wq * scale).T  # fold score scale into Wq
    wk_s = Wk.T
    wv_s = Wv.T
    mask = np.where(
        np.arange(P)[:, None] <= np.arange(P)[None, :], 0.0, NEG
    ).astype(np.float32)

    in_maps = []
    for c in range(N_CORES):
        b, hh = divmod(c, 2)
        hs = slice(hh * DLOC, (hh + 1) * DLOC)
        in_maps.append(
            {
                "xq": np.ascontiguousarray(q[b].T),
                "xk": np.ascontiguousarray(k[b].T),
                "xv": np.ascontiguousarray(v[b].T),
                "wq": np.ascontiguousarray(wq_s[:, hs]),
                "wk": np.ascontiguousarray(wk_s[:, hs]),
                "wv": np.ascontiguousarray(wv_s[:, hs]),
                "wo": np.ascontiguousarray(Wo[:, hs].T),
                "mask": mask,
            }
        )

    nc = _get_program()
    res = None
    for attempt in range(3):
        try:
            res = bass_utils.run_bass_kernel_spmd(
                nc, in_maps, core_ids=list(range(N_CORES))
            )
            break
        except Exception:
            # transient NRT_EXEC_UNIT_UNRECOVERABLE device wedges have been
            # observed on this fabric; retry a couple of times
            if attempt == 2:
                raise
            import time

            time.sleep(10)
    assert res is not None

    out = np.empty((B, T, DIN), np.float32)
    for b in range(B):
        out[b] = res.results[2 * b]["out"] + res.results[2 * b + 1]["out"]
    out += bo
    return out
